# revision 2
# baseline (speedup 1.0000x reference)
"""Trainium2 Bass kernel: causal multi-head self-attention with RoPE.

Problem: x[2,2048,1024], 16 heads, d_k=64, causal, RoPE(theta=1e4),
out = (softmax(rope(Q)rope(K)^T/8) V) WO^T.

Sharding (8 cores): data-parallel over batch (2) x head-parallel over
head groups (4 heads per core).  Each core computes Q/K/V projections
for its 4 heads, flash-style causal attention, and a partial output
projection over its 256 channels; the host sums the 4 partials per
batch element.

Device layouts (per core, all bf16 except PSUM):
  xt  [1024,2048]  x[b]^T (d_model on partitions), shipped pre-chunked
      in the exact SBUF layout so every DMA is contiguous.
  Qt/Kt tiles [128,2048]: 2 heads each, per head rows = [32 even-dim,
      32 odd-dim] (host permutes W_Q/W_K columns) so RoPE is pure
      row-block ops; scores are permutation-invariant.
  V   [128,16,4,65]: natural [s,d] layout per 128-row s-block, 65th
      column of ones => P@[V|1] yields softmax denominators for free.
  scores computed transposed ([keys,queries]) so P^T feeds P@V with the
      contraction (keys) on partitions.  Causal masking: an identity
      matmul preloads -1e5 into the key>query region of the scores psum
      (keeps the mask off the Vector engine / out of the exp->PV chain);
      PV skips fully-masked leading columns of diagonal key blocks.
  softmax normalization is deferred: unnormalized head outputs plus the
      denominator rows are staged, then one fast-reciprocal + an
      indicator-matmul broadcast rescales everything at the tail,
      pipelined per query-slice with the output projection.
"""

import os
import sys

for _p in ("/opt/trn_rl_repo",):
    if _p not in sys.path:
        sys.path.insert(0, _p)

import numpy as np
import ml_dtypes

BF16 = ml_dtypes.bfloat16

D = 1024
S = 2048
H = 16
DK = 64
HPC = 4          # heads per core
NCORES = 8
THETA = 10000.0

_COMPILED = {}


def _build_nc():
    import concourse.bass as bass  # noqa: F401
    import concourse.bacc as bacc
    import concourse.mybir as mybir
    import concourse.tile as tile

    bf16 = mybir.dt.bfloat16
    f32 = mybir.dt.float32
    Exp = mybir.ActivationFunctionType.Exp

    nc = bacc.Bacc(
        "TRN2", target_bir_lowering=False, debug=False, num_devices=NCORES
    )
    xt_d = nc.declare_dram_parameter("xt", [4, 128, 8, 512], bf16, isOutput=False)
    wq_d = nc.declare_dram_parameter("wq", [128, 8, 256], bf16, isOutput=False)
    wk_d = nc.declare_dram_parameter("wk", [128, 8, 256], bf16, isOutput=False)
    wv_d = nc.declare_dram_parameter("wv", [128, 8, 256], bf16, isOutput=False)
    wo_d = nc.declare_dram_parameter("wo", [128, 2, D], bf16, isOutput=False)
    cos_d = nc.declare_dram_parameter("cosb", [128, S], bf16, isOutput=False)
    sin_d = nc.declare_dram_parameter("sinb", [128, S], bf16, isOutput=False)
    msk_d = nc.declare_dram_parameter("msk", [128, 4, 512], bf16, isOutput=False)
    eye_d = nc.declare_dram_parameter("eye", [128, 128], bf16, isOutput=False)
    ind_d = nc.declare_dram_parameter("ind", [40, 4, 128], bf16, isOutput=False)
    out_d = nc.declare_dram_parameter("out", [S, D], bf16, isOutput=True)

    with tile.TileContext(nc) as tc:
        with tc.tile_pool(name="const", bufs=1) as const:
            x_sb = const.tile([128, 8, S], bf16)
            wq_sb = const.tile([128, 8, 256], bf16)
            wk_sb = const.tile([128, 8, 256], bf16)
            wv_sb = const.tile([128, 8, 256], bf16)
            wo_sb = const.tile([128, 2, D], bf16)
            cos_sb = const.tile([128, S], bf16)
            sin_sb = const.tile([128, S], bf16)
            msk_sb = const.tile([128, 4, 512], bf16)
            eye_sb = const.tile([128, 128], bf16)
            ind_sb = const.tile([40, 4, 128], bf16)
            v_sb = const.tile([128, 16, 4, 65], bf16)
            qraw = [const.tile([128, S], bf16, name=f"qraw{i}") for i in range(2)]
            kraw = [const.tile([128, S], bf16, name=f"kraw{i}") for i in range(2)]
            qrot = [const.tile([128, S], bf16, name=f"qrot{i}") for i in range(2)]
            krot = [const.tile([128, S], bf16, name=f"krot{i}") for i in range(2)]
            at = [const.tile([128, S], bf16, name=f"at{i}") for i in range(2)]

            # x slices stream on the HW-DGE (sync) queue; everything else is
            # issued in parallel from the gpsimd queue
            for nsl in range(4):
                nc.sync.dma_start(
                    x_sb[:, :, nsl * 512:(nsl + 1) * 512], xt_d[nsl]
                )
            nc.gpsimd.dma_start(wq_sb[:], wq_d[:])
            nc.gpsimd.dma_start(wk_sb[:], wk_d[:])
            nc.gpsimd.dma_start(wv_sb[:], wv_d[:])
            nc.gpsimd.dma_start(cos_sb[:], cos_d[:])
            nc.gpsimd.dma_start(sin_sb[:], sin_d[:])
            nc.gpsimd.dma_start(msk_sb[:], msk_d[:])
            nc.gpsimd.dma_start(eye_sb[:], eye_d[:])
            nc.gpsimd.dma_start(ind_sb[:], ind_d[:])
            nc.gpsimd.dma_start(wo_sb[:], wo_d[:])
            nc.vector.memset(v_sb[:, :, :, 64:65], 1.0)

            # ---- phase 1: Q/K/V projections + RoPE ----
            with tc.tile_pool(name="pj", bufs=4, space="PSUM") as pjp, \
                 tc.tile_pool(name="pvps", bufs=2, space="PSUM") as pvps:
                for nsl in range(4):
                    for w_sb, raw in ((wq_sb, qraw), (wk_sb, kraw)):
                        for ot in range(2):
                            ps = pjp.tile([128, 512], f32, tag="pj", name="pj")
                            for c in range(8):
                                nc.tensor.matmul(
                                    ps[:],
                                    w_sb[:, c, ot * 128:(ot + 1) * 128],
                                    x_sb[:, c, nsl * 512:(nsl + 1) * 512],
                                    start=(c == 0), stop=(c == 7),
                                )
                            nc.vector.tensor_copy(
                                raw[ot][:, nsl * 512:(nsl + 1) * 512], ps[:]
                            )
                for sb in range(16):
                    ps = pvps.tile([128, 256], f32, tag="pv", name="pv")
                    for c in range(8):
                        nc.tensor.matmul(
                            ps[:],
                            x_sb[:, c, sb * 128:(sb + 1) * 128],
                            wv_sb[:, c, :],
                            start=(c == 0), stop=(c == 7),
                        )
                    nc.vector.tensor_copy(
                        v_sb[:, sb, :, 0:64],
                        ps[:].rearrange("p (h d) -> p h d", h=4),
                    )
                with tc.tile_pool(name="rope", bufs=2) as rp:
                    for raw, rot in ((qraw, qrot), (kraw, krot)):
                        for ot in range(2):
                            sw = rp.tile([128, S], bf16, tag="sw", name="sw")
                            t1 = rp.tile([128, S], bf16, tag="t1", name="t1")
                            for blk in range(4):
                                src = blk ^ 1
                                nc.sync.dma_start(
                                    sw[blk * 32:(blk + 1) * 32, :],
                                    raw[ot][src * 32:(src + 1) * 32, :],
                                )
                            nc.vector.tensor_mul(t1[:], raw[ot][:], cos_sb[:])
                            nc.vector.tensor_mul(sw[:], sw[:], sin_sb[:])
                            nc.vector.tensor_add(rot[ot][:], t1[:], sw[:])

            # ---- phase 2: causal attention (scores transposed) ----
            den_sb = const.tile([40, 512], bf16)
            rc = const.tile([40, 512], f32)
            rcb = const.tile([40, 512], bf16)
            atn = [const.tile([128, 4, 512], bf16, name=f"atn{i}")
                   for i in range(2)]
            with tc.tile_pool(name="ps_s", bufs=3, space="PSUM") as psc, \
                 tc.tile_pool(name="ps_o", bufs=2, space="PSUM") as pso, \
                 tc.tile_pool(name="pp", bufs=3) as ppool, \
                 tc.tile_pool(name="nrm", bufs=3) as nrm:
                for h in range(HPC):
                    ot, hl = divmod(h, 2)
                    qr, kr = qrot[ot], krot[ot]
                    r0 = hl * 64
                    for j in range(4):
                        nkb = 4 * (j + 1)
                        po = pso.tile([65, 512], f32, tag="po", name="po")
                        for g0 in range(0, nkb, 2):
                            G = min(2, nkb - g0)
                            sp = psc.tile([128, 1024], f32, tag="sc", name="sp")
                            pt = ppool.tile([128, 1024], bf16, tag="pt", name="pt")
                            for i in range(G):
                                kb = g0 + i
                                dg = kb - 4 * j
                                if dg >= 0:
                                    # causal mask: preload psum with -1e5 in
                                    # the key>query region via identity matmul
                                    nc.tensor.matmul(
                                        sp[:, i * 512:(i + 1) * 512],
                                        eye_sb[:],
                                        msk_sb[:, dg, :],
                                        start=True, stop=False,
                                    )
                                nc.tensor.matmul(
                                    sp[:, i * 512:(i + 1) * 512],
                                    kr[r0:r0 + 64, kb * 128:(kb + 1) * 128],
                                    qr[r0:r0 + 64, j * 512:(j + 1) * 512],
                                    start=(dg < 0), stop=True,
                                )
                            nc.scalar.activation(
                                pt[:, 0:G * 512], sp[:, 0:G * 512], Exp, scale=0.125
                            )
                            for i in range(G):
                                kb = g0 + i
                                dg = kb - 4 * j
                                # cols < dg*128 of a diagonal block are fully
                                # masked (exactly 0 after exp): PV skips them
                                c0 = dg * 128 if dg > 0 else 0
                                nc.tensor.matmul(
                                    po[:, c0:512],
                                    v_sb[:, kb, h, 0:65],
                                    pt[:, i * 512 + c0:(i + 1) * 512],
                                    start=(kb == 0), stop=(kb == nkb - 1),
                                )
                        # stage unnormalized out + denominator, release po fast
                        tm = nrm.tile([65, 512], bf16, tag="tm", name="tm")
                        nc.vector.tensor_copy(tm[:], po[:])
                        nc.sync.dma_start(
                            at[ot][r0:r0 + 64, j * 512:(j + 1) * 512], tm[0:64, :]
                        )
                        dr = ot * 32 + hl * 4 + j
                        nc.sync.dma_start(den_sb[dr:dr + 1, :], tm[64:65, :])

            # ---- tail: normalization + output projection, pipelined per jsl ----
            with tc.tile_pool(name="ps_r", bufs=2, space="PSUM") as psr, \
                 tc.tile_pool(name="ps_f", bufs=4, space="PSUM") as psf, \
                 tc.tile_pool(name="ost", bufs=4) as ost:
                denf = ost.tile([40, 512], f32, tag="denf", name="denf")
                nc.vector.tensor_copy(denf[:], den_sb[:])
                nc.vector.reciprocal_approx_fast(rc[:], denf[:])
                nc.vector.tensor_copy(rcb[:], rc[:])
                for jsl in range(4):
                    for ot in range(2):
                        rbp = psr.tile([128, 512], f32, tag="rb", name="rb")
                        nc.tensor.matmul(
                            rbp[:], ind_sb[ot * 32:ot * 32 + 8, jsl, :],
                            rcb[ot * 32:ot * 32 + 8, :], start=True, stop=True,
                        )
                        nc.vector.tensor_mul(
                            atn[ot][:, jsl, :],
                            at[ot][:, jsl * 512:(jsl + 1) * 512],
                            rbp[:],
                        )
                    for sbi in range(4):
                        sb = jsl * 4 + sbi
                        for osl in range(2):
                            pf = psf.tile([128, 512], f32, tag="pf", name="pf")
                            for ich in range(2):
                                nc.tensor.matmul(
                                    pf[:],
                                    atn[ich][:, jsl, sbi * 128:(sbi + 1) * 128],
                                    wo_sb[:, ich, osl * 512:(osl + 1) * 512],
                                    start=(ich == 0), stop=(ich == 1),
                                )
                            ob = ost.tile([128, 512], bf16, tag="ob", name="ob")
                            nc.scalar.copy(ob[:], pf[:])
                            nc.sync.dma_start(
                                out_d[sb * 128:(sb + 1) * 128,
                                      osl * 512:(osl + 1) * 512],
                                ob[:],
                            )
    nc.compile()
    return nc


def _host_prep(x, token_positions, WQ, WK, WV, WO):
    """Build the 8 per-core input maps."""
    pos = np.asarray(token_positions).astype(np.float32)
    k = np.arange(DK // 2, dtype=np.float32)
    inv_freq = 1.0 / (THETA ** (2.0 * k / DK))
    ang = pos[:, None] * inv_freq[None, :]          # [S, 32]
    c32 = np.cos(ang).T.astype(np.float32)          # [32, S]
    s32 = np.sin(ang).T.astype(np.float32)
    cosb = np.tile(c32, (4, 1)).astype(BF16)        # [128, S]
    sinb = np.concatenate([-s32, s32, -s32, s32], axis=0).astype(BF16)
    # causal masks for the 4 diagonal key-blocks of a 512-query slice
    kk = np.arange(128)[:, None, None]
    dd = np.arange(4)[None, :, None]
    qq = np.arange(512)[None, None, :]
    msk = np.where(dd * 128 + kk <= qq, 0.0, -1e5).astype(BF16)  # [128, 4, 512]
    eye = np.eye(128, dtype=np.float32).astype(BF16)
    # indicator matrices for denominator broadcast:
    # ind[i, jsl, r] = 1 iff i == (r//64)*4 + jsl  (same for both head pairs)
    ind = np.zeros((40, 4, 128), dtype=np.float32)
    for jsl in range(4):
        for r in range(128):
            ind[(r // 64) * 4 + jsl, jsl, r] = 1.0
            ind[32 + (r // 64) * 4 + jsl, jsl, r] = 1.0
    ind = ind.astype(BF16)

    perm = np.concatenate([np.arange(0, DK, 2), np.arange(1, DK, 2)])  # evens,odds

    in_maps = []
    for core in range(NCORES):
        b, hg = divmod(core, 4)
        ch0 = hg * 256
        qk_rows = np.concatenate([ch0 + hl * 64 + perm for hl in range(HPC)])
        def dev_w(w):  # [D, M] -> [128, 8, M] (contraction chunks)
            return np.ascontiguousarray(
                w.reshape(8, 128, -1).transpose(1, 0, 2)
            ).astype(BF16)

        xt = np.asarray(x[b]).T                       # [D, S]
        xt4 = np.ascontiguousarray(
            xt.reshape(8, 128, 4, 512).transpose(2, 1, 0, 3)
        ).astype(BF16)                                # [4, 128, 8, 512]
        in_maps.append({
            "xt": xt4,
            "wq": dev_w(np.asarray(WQ)[qk_rows, :].T),
            "wk": dev_w(np.asarray(WK)[qk_rows, :].T),
            "wv": dev_w(np.asarray(WV)[ch0:ch0 + 256, :].T),
            "wo": np.ascontiguousarray(
                np.asarray(WO)[:, ch0:ch0 + 256].T.reshape(2, 128, D)
                .transpose(1, 0, 2)
            ).astype(BF16),
            "cosb": cosb,
            "sinb": sinb,
            "msk": msk,
            "eye": eye,
            "ind": ind,
        })
    return in_maps


LAST_EXEC_NS = None
LAST_RES = None


def kernel(x, token_positions, WQ, WK, WV, WO):
    global LAST_EXEC_NS, LAST_RES
    from concourse.bass_utils import run_bass_kernel_spmd

    if "nc" not in _COMPILED:
        _COMPILED["nc"] = _build_nc()
    nc = _COMPILED["nc"]

    in_maps = _host_prep(x, token_positions, WQ, WK, WV, WO)
    res = run_bass_kernel_spmd(nc, in_maps, list(range(NCORES)))
    LAST_RES = res
    LAST_EXEC_NS = res.exec_time_ns

    out = np.zeros((2, S, D), dtype=np.float32)
    for core in range(NCORES):
        out[core // 4] += np.asarray(res.results[core]["out"], dtype=np.float32)
    return out



# revision 16
# speedup vs baseline: 1.0353x; 1.0353x over previous
"""Trainium2 Bass kernel: causal multi-head self-attention with RoPE.

Problem: x[2,2048,1024], 16 heads, d_k=64, causal, RoPE(theta=1e4),
out = (softmax(rope(Q)rope(K)^T/8) V) WO^T.

Sharding (8 cores): data-parallel over batch (2) x head-parallel over
head groups (4 heads per core).  Each core computes Q/K/V projections
for its 4 heads, flash-style causal attention, and a partial output
projection over its 256 channels; the host sums the 4 partials per
batch element.

v2 design notes (vs baseline):
  - x staged as 4 independent SBUF tiles so the first projection matmul
    only waits on the first 1MB DMA slice, not all of x.
  - phase 2 is jsl-major / head-minor with a 1-group software pipeline:
    scores(g+1) issue before PV(g) so the PE never stalls behind the
    Act engine's exp.
  - no -1e5 mask preload matmuls: the causal triangle of each diagonal
    128x128 block is zeroed multiplicatively post-exp on the (otherwise
    idle) Pool engine; fully-masked leading columns are skipped in the
    scores matmul, the exp, and PV.
  - the output projection + softmax normalization for query slice jsl
    is interleaved into the scores/PV stream of jsl+1 instead of
    running as a serial tail.
  - psum->sbuf copies ride on Pool, freeing DVE for RoPE / staging.
"""

import os
import sys

for _p in ("/opt/trn_rl_repo",):
    if _p not in sys.path:
        sys.path.insert(0, _p)

import numpy as np
import ml_dtypes

BF16 = ml_dtypes.bfloat16

D = 1024
S = 2048
H = 16
DK = 64
HPC = 4          # heads per core
NCORES = 8
THETA = 10000.0

_COMPILED = {}


def _build_nc():
    import concourse.bass as bass  # noqa: F401
    import concourse.bacc as bacc
    import concourse.mybir as mybir
    import concourse.tile as tile

    bf16 = mybir.dt.bfloat16
    f32 = mybir.dt.float32
    Exp = mybir.ActivationFunctionType.Exp

    nc = bacc.Bacc(
        "TRN2", target_bir_lowering=False, debug=False, num_devices=NCORES
    )
    xt_d = nc.declare_dram_parameter("xt", [4, 128, 8, 512], bf16, isOutput=False)
    wq_d = nc.declare_dram_parameter("wq", [128, 8, 256], bf16, isOutput=False)
    wk_d = nc.declare_dram_parameter("wk", [128, 8, 256], bf16, isOutput=False)
    wv_d = nc.declare_dram_parameter("wv", [128, 8, 256], bf16, isOutput=False)
    wo_d = nc.declare_dram_parameter("wo", [128, 2, D], bf16, isOutput=False)
    cos_d = nc.declare_dram_parameter("cosb", [128, S], bf16, isOutput=False)
    sin_d = nc.declare_dram_parameter("sinb", [128, S], bf16, isOutput=False)
    tri_d = nc.declare_dram_parameter("tri", [128, 128], bf16, isOutput=False)
    ind_d = nc.declare_dram_parameter("ind2", [2, 128], bf16, isOutput=False)
    out_d = nc.declare_dram_parameter("out", [S, D], bf16, isOutput=True)

    with tile.TileContext(nc) as tc:
        with tc.tile_pool(name="const", bufs=1) as const:
            x_sb = [const.tile([128, 8, 512], bf16, name=f"x{i}") for i in range(4)]
            wq_sb = const.tile([128, 8, 256], bf16)
            wk_sb = const.tile([128, 8, 256], bf16)
            wv_sb = const.tile([128, 8, 256], bf16)
            wo_sb = const.tile([128, 2, D], bf16)
            cos_sb = const.tile([128, S], bf16)
            sin_sb = const.tile([128, S], bf16)
            tri_sb = const.tile([128, 128], bf16)
            ind_sb = const.tile([2, 128], bf16)
            v_sb = const.tile([128, 16, 4, 65], bf16)
            qraw = [const.tile([128, S], bf16, name=f"qraw{i}") for i in range(2)]
            kraw = [const.tile([128, S], bf16, name=f"kraw{i}") for i in range(2)]
            qrot = [const.tile([128, S], bf16, name=f"qrot{i}") for i in range(2)]
            krot = [const.tile([128, S], bf16, name=f"krot{i}") for i in range(2)]
            # unnormalized head outputs, one tile per (ot, jsl)
            atj = [[const.tile([128, 512], bf16, name=f"at{o}_{j}")
                    for j in range(4)] for o in range(2)]
            # normalized, one tile per (ot, jsl)
            atn = [[const.tile([128, 512], bf16, name=f"an{o}_{j}")
                    for j in range(4)] for o in range(2)]
            den_sb = const.tile([2, 8, 512], bf16)  # [hl, jsl*2+ot, q]

            # x slices stream on the HW-DGE (sync) queue; weights on the
            # gpsimd SW-DGE queue in first-needed order.
            for nsl in range(4):
                nc.sync.dma_start(x_sb[nsl][:], xt_d[nsl])
            nc.gpsimd.dma_start(wq_sb[:], wq_d[:])
            nc.gpsimd.dma_start(wk_sb[:], wk_d[:])
            nc.gpsimd.dma_start(cos_sb[:], cos_d[:])
            nc.gpsimd.dma_start(sin_sb[:], sin_d[:])
            nc.gpsimd.dma_start(wv_sb[:], wv_d[:])
            nc.gpsimd.dma_start(tri_sb[:], tri_d[:])
            nc.gpsimd.dma_start(ind_sb[:], ind_d[:])
            nc.gpsimd.dma_start(wo_sb[:], wo_d[:])
            nc.vector.memset(v_sb[:, :, :, 64:65], 1.0)

            # ---- phase 1: Q/K/V projections + RoPE ----
            with tc.tile_pool(name="pj", bufs=1, space="PSUM") as pjp, \
                 tc.tile_pool(name="rope", bufs=1) as rp:

                def rope_pair(raw, rot, ot):
                    sw = rp.tile([128, S], bf16, tag="sw", name="sw", bufs=2)
                    t1 = rp.tile([128, S], bf16, tag="t1", name="t1", bufs=2)
                    for blk in range(4):
                        src = blk ^ 1
                        nc.sync.dma_start(
                            sw[blk * 32:(blk + 1) * 32, :],
                            raw[ot][src * 32:(src + 1) * 32, :],
                        )
                    nc.vector.tensor_mul(t1[:], raw[ot][:], cos_sb[:])
                    nc.vector.tensor_mul(sw[:], sw[:], sin_sb[:])
                    nc.vector.tensor_add(rot[ot][:], t1[:], sw[:])

                for ot in range(2):
                    for w_sb, raw in ((wq_sb, qraw), (wk_sb, kraw)):
                        for nsl in range(4):
                            ps = pjp.tile([128, 512], f32, tag="pj", name="pj",
                                          bufs=4)
                            for c in range(8):
                                nc.tensor.matmul(
                                    ps[:],
                                    w_sb[:, c, ot * 128:(ot + 1) * 128],
                                    x_sb[nsl][:, c, :],
                                    start=(c == 0), stop=(c == 7),
                                )
                            nc.scalar.copy(
                                raw[ot][:, nsl * 512:(nsl + 1) * 512], ps[:]
                            )
                    # rope for this ot as soon as its raws exist
                    rope_pair(qraw, qrot, ot)
                    rope_pair(kraw, krot, ot)
                for sb in range(16):
                    ps = pjp.tile([128, 256], f32, tag="pv", name="pv", bufs=3)
                    for c in range(8):
                        nc.tensor.matmul(
                            ps[:],
                            x_sb[sb // 4][:, c, (sb % 4) * 128:(sb % 4 + 1) * 128],
                            wv_sb[:, c, :],
                            start=(c == 0), stop=(c == 7),
                        )
                    nc.scalar.copy(
                        v_sb[:, sb, :, 0:64],
                        ps[:].rearrange("p (h d) -> p h d", h=4),
                    )

            # ---- phase 2: causal attention, jsl-major, software-pipelined,
            #      with the previous jsl's normalization+output projection
            #      interleaved into the stream ----
            with tc.tile_pool(name="ps_s", bufs=1, space="PSUM") as psc, \
                 tc.tile_pool(name="ps_o", bufs=1, space="PSUM") as pso, \
                 tc.tile_pool(name="ps_f", bufs=1, space="PSUM") as psf, \
                 tc.tile_pool(name="pp", bufs=1) as ppool, \
                 tc.tile_pool(name="nrm", bufs=1) as nrm:

                def make_unit(jsl, h, g0, nkb):
                    ot, hl = divmod(h, 2)
                    r0 = hl * 64
                    qr, kr = qrot[ot], krot[ot]
                    state = {}

                    def emit_scores():
                        sp = psc.tile([128, 1024], f32, tag="sc", name="sp",
                                      bufs=2)
                        pt = ppool.tile([128, 1024], bf16, tag="pt", name="pt",
                                        bufs=3)
                        state["sp"], state["pt"] = sp, pt
                        dgs = []
                        for i in range(2):
                            kb = g0 + i
                            dg = kb - 4 * jsl
                            c0 = dg * 128 if dg > 0 else 0
                            nc.tensor.matmul(
                                sp[:, i * 512 + c0:(i + 1) * 512],
                                kr[r0:r0 + 64, kb * 128:(kb + 1) * 128],
                                qr[r0:r0 + 64,
                                   jsl * 512 + c0:(jsl + 1) * 512],
                                start=True, stop=True,
                            )
                            dgs.append(dg)
                        # exp: skip fully-masked leading columns where the
                        # block's live range starts at >=256
                        if dgs[0] >= 2:
                            for i in range(2):
                                c0 = dgs[i] * 128
                                nc.scalar.activation(
                                    pt[:, i * 512 + c0:(i + 1) * 512],
                                    sp[:, i * 512 + c0:(i + 1) * 512],
                                    Exp, scale=0.125,
                                )
                        else:
                            nc.scalar.activation(
                                pt[:, 0:1024], sp[:, 0:1024], Exp, scale=0.125
                            )
                        # zero the causal triangle of diagonal blocks (Pool)
                        for i in range(2):
                            dg = g0 + i - 4 * jsl
                            if 0 <= dg <= 3:
                                a = i * 512 + dg * 128
                                nc.gpsimd.tensor_mul(
                                    pt[:, a:a + 128], pt[:, a:a + 128],
                                    tri_sb[:],
                                )

                    def emit_pv(po):
                        pt = state["pt"]
                        for i in range(2):
                            kb = g0 + i
                            dg = kb - 4 * jsl
                            c0 = dg * 128 if dg > 0 else 0
                            nc.tensor.matmul(
                                po[:, c0:512],
                                v_sb[:, kb, h, 0:65],
                                pt[:, i * 512 + c0:(i + 1) * 512],
                                start=(kb == 0), stop=(kb == nkb - 1),
                            )

                    return emit_scores, emit_pv

                def emit_stage(jsl, h, po):
                    ot, hl = divmod(h, 2)
                    r0 = hl * 64
                    tm = nrm.tile([65, 512], bf16, tag="tm", name="tm", bufs=3)
                    nc.vector.tensor_copy(tm[:], po[:])
                    nc.sync.dma_start(atj[ot][jsl][r0:r0 + 64, :], tm[0:64, :])
                    nc.sync.dma_start(
                        den_sb[hl:hl + 1, jsl * 2 + ot, :], tm[64:65, :])

                def make_tail(jsl):
                    """Normalization + output projection for jsl as a list of
                    closures to interleave into the next jsl's PE stream."""
                    ops = []

                    def t_norm(ot):
                        denf = nrm.tile([2, 512], f32, tag="denf",
                                        name="denf", bufs=2)
                        rc2 = nrm.tile([2, 512], f32, tag="rc2", name="rc2",
                                       bufs=2)
                        rcb = nrm.tile([2, 512], bf16, tag="rcb", name="rcb",
                                       bufs=2)
                        nc.vector.tensor_copy(
                            denf[:], den_sb[0:2, jsl * 2 + ot, :])
                        nc.vector.reciprocal_approx_fast(rc2[:], denf[:])
                        nc.vector.tensor_copy(rcb[:], rc2[:])
                        rbp = psf.tile([128, 512], f32, tag="pf", name="rb",
                                       bufs=2)
                        nc.tensor.matmul(
                            rbp[:], ind_sb[0:2, :], rcb[0:2, :],
                            start=True, stop=True,
                        )
                        nc.vector.tensor_mul(
                            atn[ot][jsl][:], atj[ot][jsl][:], rbp[:])
                    ops.append(lambda: t_norm(0))
                    ops.append(lambda: t_norm(1))

                    def t_proj(sbi, osl):
                        pf = psf.tile([128, 512], f32, tag="pf", name="pf",
                                      bufs=2)
                        for ich in range(2):
                            nc.tensor.matmul(
                                pf[:],
                                atn[ich][jsl][:, sbi * 128:(sbi + 1) * 128],
                                wo_sb[:, ich, osl * 512:(osl + 1) * 512],
                                start=(ich == 0), stop=(ich == 1),
                            )
                        ob = nrm.tile([128, 512], bf16, tag="ob", name="ob",
                                      bufs=3)
                        nc.vector.tensor_copy(ob[:], pf[:])
                        sb = jsl * 4 + sbi
                        nc.sync.dma_start(
                            out_d[sb * 128:(sb + 1) * 128,
                                  osl * 512:(osl + 1) * 512],
                            ob[:],
                        )
                    for sbi in range(4):
                        for osl in range(2):
                            ops.append(lambda s=sbi, o=osl: t_proj(s, o))
                    return ops

                tail_ops = []
                for jsl in range(4):
                    nkb = 4 * (jsl + 1)
                    units = []
                    po_of_head = {}
                    for h in range(HPC):
                        po = pso.tile([65, 512], f32, tag="po",
                                      name=f"po{jsl}{h}", bufs=2)
                        po_of_head[h] = po
                        for g0 in range(0, nkb, 2):
                            units.append(
                                (h, g0, *make_unit(jsl, h, g0, nkb)))
                    # software pipeline: scores(i+1) issues before PV(i);
                    # tail ops of jsl-1 sprinkle into the stream
                    ntail = len(tail_ops)
                    ti = 0
                    prev = None
                    for ui, (h, g0, es, epv) in enumerate(units):
                        es()
                        if ntail and ui % 1 == 0 and ti < ntail \
                                and ui >= (ti + 1) * len(units) // (ntail + 1):
                            tail_ops[ti]()
                            ti += 1
                        if prev is not None:
                            ph, pg0, pes, pepv = prev
                            pepv(po_of_head[ph])
                            if pg0 + 2 >= nkb:
                                emit_stage(jsl, ph, po_of_head[ph])
                        prev = (h, g0, es, epv)
                    while ti < ntail:
                        tail_ops[ti]()
                        ti += 1
                    ph, pg0, pes, pepv = prev
                    pepv(po_of_head[ph])
                    emit_stage(jsl, ph, po_of_head[ph])
                    tail_ops = make_tail(jsl)
                # final jsl's tail
                for op in tail_ops:
                    op()
    nc.compile()
    return nc


def _host_prep(x, token_positions, WQ, WK, WV, WO):
    """Build the 8 per-core input maps."""
    pos = np.asarray(token_positions).astype(np.float32)
    k = np.arange(DK // 2, dtype=np.float32)
    inv_freq = 1.0 / (THETA ** (2.0 * k / DK))
    ang = pos[:, None] * inv_freq[None, :]          # [S, 32]
    c32 = np.cos(ang).T.astype(np.float32)          # [32, S]
    s32 = np.sin(ang).T.astype(np.float32)
    cosb = np.tile(c32, (4, 1)).astype(BF16)        # [128, S]
    sinb = np.concatenate([-s32, s32, -s32, s32], axis=0).astype(BF16)
    # 0/1 lower-triangle for zeroing the causal triangle of diagonal blocks
    kk = np.arange(128)[:, None]
    qq = np.arange(128)[None, :]
    tri = (qq >= kk).astype(np.float32).astype(BF16)        # [128, 128]
    # denominator-broadcast indicator: ind2[hl, r] = 1 iff r//64 == hl
    ind2 = np.zeros((2, 128), dtype=np.float32)
    ind2[0, 0:64] = 1.0
    ind2[1, 64:128] = 1.0
    ind2 = ind2.astype(BF16)

    perm = np.concatenate([np.arange(0, DK, 2), np.arange(1, DK, 2)])  # evens,odds

    in_maps = []
    for core in range(NCORES):
        b, hg = divmod(core, 4)
        ch0 = hg * 256
        qk_rows = np.concatenate([ch0 + hl * 64 + perm for hl in range(HPC)])
        def dev_w(w):  # [D, M] -> [128, 8, M] (contraction chunks)
            return np.ascontiguousarray(
                w.reshape(8, 128, -1).transpose(1, 0, 2)
            ).astype(BF16)

        xt = np.asarray(x[b]).T                       # [D, S]
        xt4 = np.ascontiguousarray(
            xt.reshape(8, 128, 4, 512).transpose(2, 1, 0, 3)
        ).astype(BF16)                                # [4, 128, 8, 512]
        in_maps.append({
            "xt": xt4,
            "wq": dev_w(np.asarray(WQ)[qk_rows, :].T),
            "wk": dev_w(np.asarray(WK)[qk_rows, :].T),
            "wv": dev_w(np.asarray(WV)[ch0:ch0 + 256, :].T),
            "wo": np.ascontiguousarray(
                np.asarray(WO)[:, ch0:ch0 + 256].T.reshape(2, 128, D)
                .transpose(1, 0, 2)
            ).astype(BF16),
            "cosb": cosb,
            "sinb": sinb,
            "tri": tri,
            "ind2": ind2,
        })
    return in_maps


LAST_EXEC_NS = None
LAST_RES = None


def kernel(x, token_positions, WQ, WK, WV, WO):
    global LAST_EXEC_NS, LAST_RES
    from concourse.bass_utils import run_bass_kernel_spmd

    if "nc" not in _COMPILED:
        _COMPILED["nc"] = _build_nc()
    nc = _COMPILED["nc"]

    in_maps = _host_prep(x, token_positions, WQ, WK, WV, WO)
    res = run_bass_kernel_spmd(nc, in_maps, list(range(NCORES)))
    LAST_RES = res
    LAST_EXEC_NS = res.exec_time_ns

    out = np.zeros((2, S, D), dtype=np.float32)
    for core in range(NCORES):
        out[core // 4] += np.asarray(res.results[core]["out"], dtype=np.float32)
    return out


# revision 20
# speedup vs baseline: 1.1351x; 1.0965x over previous
"""Trainium2 Bass kernel: causal multi-head self-attention with RoPE.

Problem: x[2,2048,1024], 16 heads, d_k=64, causal, RoPE(theta=1e4),
out = (softmax(rope(Q)rope(K)^T/8) V) WO^T.

Sharding (8 cores): data-parallel over batch (2) x head-parallel over
head groups (4 heads per core).  Each core computes Q/K/V projections
for its 4 heads, flash-style causal attention, and a partial output
projection over its 256 channels; the host sums the 4 partials per
batch element.

v2 design notes (vs baseline):
  - x staged as 4 independent SBUF tiles so the first projection matmul
    only waits on the first 1MB DMA slice, not all of x.
  - phase 2 is jsl-major / head-minor with a 1-group software pipeline:
    scores(g+1) issue before PV(g) so the PE never stalls behind the
    Act engine's exp.
  - no -1e5 mask preload matmuls: the causal triangle of each diagonal
    128x128 block is zeroed multiplicatively post-exp on the (otherwise
    idle) Pool engine; fully-masked leading columns are skipped in the
    scores matmul, the exp, and PV.
  - the output projection + softmax normalization for query slice jsl
    is interleaved into the scores/PV stream of jsl+1 instead of
    running as a serial tail.
  - psum->sbuf copies ride on Pool, freeing DVE for RoPE / staging.
"""

import os
import sys

for _p in ("/opt/trn_rl_repo",):
    if _p not in sys.path:
        sys.path.insert(0, _p)

import numpy as np
import ml_dtypes

BF16 = ml_dtypes.bfloat16

D = 1024
S = 2048
H = 16
DK = 64
HPC = 4          # heads per core
NCORES = 8
THETA = 10000.0

_COMPILED = {}


def _build_nc():
    import concourse.bass as bass  # noqa: F401
    import concourse.bacc as bacc
    import concourse.mybir as mybir
    import concourse.tile as tile

    bf16 = mybir.dt.bfloat16
    f32 = mybir.dt.float32
    Exp = mybir.ActivationFunctionType.Exp

    nc = bacc.Bacc(
        "TRN2", target_bir_lowering=False, debug=False, num_devices=NCORES
    )
    xt_d = nc.declare_dram_parameter("xt", [4, 128, 8, 512], bf16, isOutput=False)
    wq_d = nc.declare_dram_parameter("wq", [128, 8, 256], bf16, isOutput=False)
    wk_d = nc.declare_dram_parameter("wk", [128, 8, 256], bf16, isOutput=False)
    wv_d = nc.declare_dram_parameter("wv", [128, 8, 256], bf16, isOutput=False)
    wo_d = nc.declare_dram_parameter("wo", [128, 2, D], bf16, isOutput=False)
    cos_d = nc.declare_dram_parameter("cosb", [128, S], bf16, isOutput=False)
    sin_d = nc.declare_dram_parameter("sinb", [128, S], bf16, isOutput=False)
    tri_d = nc.declare_dram_parameter("tri", [128, 128], bf16, isOutput=False)
    ind_d = nc.declare_dram_parameter("ind2", [2, 128], bf16, isOutput=False)
    out_d = nc.declare_dram_parameter("out", [S, D], bf16, isOutput=True)

    with tile.TileContext(nc) as tc:
        with tc.tile_pool(name="const", bufs=1) as const:
            x_sb = [const.tile([128, 8, 512], bf16, name=f"x{i}") for i in range(4)]
            wq_sb = const.tile([128, 8, 256], bf16)
            wk_sb = const.tile([128, 8, 256], bf16)
            wv_sb = const.tile([128, 8, 256], bf16)
            wo_sb = const.tile([128, 2, D], bf16)
            cos_sb = const.tile([128, S], bf16)
            sin_sb = const.tile([128, S], bf16)
            tri_sb = const.tile([128, 128], bf16)
            ind_sb = const.tile([2, 128], bf16)
            v_sb = const.tile([128, 16, 4, 65], bf16)
            qraw = [const.tile([128, S], bf16, name=f"qraw{i}") for i in range(2)]
            kraw = [const.tile([128, S], bf16, name=f"kraw{i}") for i in range(2)]
            qrot = [const.tile([128, S], bf16, name=f"qrot{i}") for i in range(2)]
            krot = [const.tile([128, S], bf16, name=f"krot{i}") for i in range(2)]
            # unnormalized head outputs, one tile per (ot, jsl)
            atj = [[const.tile([128, 512], bf16, name=f"at{o}_{j}")
                    for j in range(4)] for o in range(2)]
            # normalized, one tile per (ot, jsl)
            atn = [[const.tile([128, 512], bf16, name=f"an{o}_{j}")
                    for j in range(4)] for o in range(2)]
            den_sb = const.tile([2, 8, 512], bf16)  # [hl, jsl*2+ot, q]

            # critical-path inputs in-order on the HW-DGE (sync) queue:
            # wq+wk+x0 gate the first projection chains.  Everything else
            # rides the gpsimd SW-DGE queue and shares leftover HBM bw.
            nc.sync.dma_start(wq_sb[:], wq_d[:])
            nc.sync.dma_start(wk_sb[:], wk_d[:])
            for nsl in range(4):
                nc.sync.dma_start(x_sb[nsl][:], xt_d[nsl])
            nc.gpsimd.dma_start(cos_sb[:], cos_d[:])
            nc.gpsimd.dma_start(sin_sb[:], sin_d[:])
            nc.gpsimd.dma_start(wv_sb[:], wv_d[:])
            nc.gpsimd.dma_start(tri_sb[:], tri_d[:])
            nc.gpsimd.dma_start(ind_sb[:], ind_d[:])
            nc.gpsimd.dma_start(wo_sb[:], wo_d[:])
            nc.vector.memset(v_sb[:, :, :, 64:65], 1.0)

            # ---- phase 1: Q/K/V projections + RoPE ----
            with tc.tile_pool(name="pj", bufs=1, space="PSUM") as pjp, \
                 tc.tile_pool(name="rope", bufs=1) as rp:

                def rope_pair(raw, rot, ot):
                    sw = rp.tile([128, S], bf16, tag="sw", name="sw", bufs=2)
                    t1 = rp.tile([128, S], bf16, tag="t1", name="t1", bufs=2)
                    for blk in range(4):
                        src = blk ^ 1
                        nc.sync.dma_start(
                            sw[blk * 32:(blk + 1) * 32, :],
                            raw[ot][src * 32:(src + 1) * 32, :],
                        )
                    nc.vector.tensor_mul(t1[:], raw[ot][:], cos_sb[:])
                    nc.vector.tensor_mul(sw[:], sw[:], sin_sb[:])
                    nc.vector.tensor_add(rot[ot][:], t1[:], sw[:])

                for ot in range(2):
                    # nsl-outer so both Q and K consume x slice nsl before
                    # slice nsl+1's DMA must have landed
                    for nsl in range(4):
                        for w_sb, raw in ((wq_sb, qraw), (wk_sb, kraw)):
                            ps = pjp.tile([128, 512], f32, tag="pj", name="pj",
                                          bufs=4)
                            for c in range(8):
                                nc.tensor.matmul(
                                    ps[:],
                                    w_sb[:, c, ot * 128:(ot + 1) * 128],
                                    x_sb[nsl][:, c, :],
                                    start=(c == 0), stop=(c == 7),
                                )
                            nc.scalar.copy(
                                raw[ot][:, nsl * 512:(nsl + 1) * 512], ps[:]
                            )
                    # rope for this ot as soon as its raws exist
                    rope_pair(qraw, qrot, ot)
                    rope_pair(kraw, krot, ot)
                for sb in range(16):
                    ps = pjp.tile([128, 256], f32, tag="pv", name="pv", bufs=3)
                    for c in range(8):
                        nc.tensor.matmul(
                            ps[:],
                            x_sb[sb // 4][:, c, (sb % 4) * 128:(sb % 4 + 1) * 128],
                            wv_sb[:, c, :],
                            start=(c == 0), stop=(c == 7),
                        )
                    nc.scalar.copy(
                        v_sb[:, sb, :, 0:64],
                        ps[:].rearrange("p (h d) -> p h d", h=4),
                    )

            # ---- phase 2: causal attention, jsl-major, software-pipelined,
            #      with the previous jsl's normalization+output projection
            #      interleaved into the stream ----
            with tc.tile_pool(name="ps_s", bufs=1, space="PSUM") as psc, \
                 tc.tile_pool(name="ps_o", bufs=1, space="PSUM") as pso, \
                 tc.tile_pool(name="ps_f", bufs=1, space="PSUM") as psf, \
                 tc.tile_pool(name="pp", bufs=1) as ppool, \
                 tc.tile_pool(name="nrm", bufs=1) as nrm:

                def make_unit(jsl, h, g0, nkb):
                    ot, hl = divmod(h, 2)
                    r0 = hl * 64
                    qr, kr = qrot[ot], krot[ot]
                    state = {}

                    def emit_scores():
                        sp = psc.tile([128, 1024], f32, tag="sc", name="sp",
                                      bufs=2)
                        pt = ppool.tile([128, 1024], bf16, tag="pt", name="pt",
                                        bufs=3)
                        state["sp"], state["pt"] = sp, pt
                        dgs = []
                        for i in range(2):
                            kb = g0 + i
                            dg = kb - 4 * jsl
                            c0 = dg * 128 if dg > 0 else 0
                            nc.tensor.matmul(
                                sp[:, i * 512 + c0:(i + 1) * 512],
                                kr[r0:r0 + 64, kb * 128:(kb + 1) * 128],
                                qr[r0:r0 + 64,
                                   jsl * 512 + c0:(jsl + 1) * 512],
                                start=True, stop=True,
                            )
                            dgs.append(dg)
                        # exp: skip fully-masked leading columns where the
                        # block's live range starts at >=256
                        if dgs[0] >= 2:
                            for i in range(2):
                                c0 = dgs[i] * 128
                                nc.scalar.activation(
                                    pt[:, i * 512 + c0:(i + 1) * 512],
                                    sp[:, i * 512 + c0:(i + 1) * 512],
                                    Exp, scale=0.125,
                                )
                        else:
                            nc.scalar.activation(
                                pt[:, 0:1024], sp[:, 0:1024], Exp, scale=0.125
                            )
                        # zero the causal triangle of diagonal blocks (DVE:
                        # fast sem handling keeps the exp->PV latency short)
                        for i in range(2):
                            dg = g0 + i - 4 * jsl
                            if 0 <= dg <= 3:
                                a = i * 512 + dg * 128
                                nc.vector.tensor_mul(
                                    pt[:, a:a + 128], pt[:, a:a + 128],
                                    tri_sb[:],
                                )

                    def emit_pv(po):
                        pt = state["pt"]
                        for i in range(2):
                            kb = g0 + i
                            dg = kb - 4 * jsl
                            c0 = dg * 128 if dg > 0 else 0
                            nc.tensor.matmul(
                                po[:, c0:512],
                                v_sb[:, kb, h, 0:65],
                                pt[:, i * 512 + c0:(i + 1) * 512],
                                start=(kb == 0), stop=(kb == nkb - 1),
                            )

                    return emit_scores, emit_pv

                def emit_stage(jsl, h, po):
                    ot, hl = divmod(h, 2)
                    r0 = hl * 64
                    tm = nrm.tile([65, 512], bf16, tag="tm", name="tm", bufs=3)
                    nc.vector.tensor_copy(tm[:], po[:])
                    nc.sync.dma_start(atj[ot][jsl][r0:r0 + 64, :], tm[0:64, :])
                    nc.sync.dma_start(
                        den_sb[hl:hl + 1, jsl * 2 + ot, :], tm[64:65, :])

                def t_norm(jsl, ot):
                    denf = nrm.tile([2, 512], f32, tag="denf",
                                    name="denf", bufs=2)
                    rc2 = nrm.tile([2, 512], f32, tag="rc2", name="rc2",
                                   bufs=2)
                    rcb = nrm.tile([2, 512], bf16, tag="rcb", name="rcb",
                                   bufs=2)
                    nc.vector.tensor_copy(
                        denf[:], den_sb[0:2, jsl * 2 + ot, :])
                    nc.vector.reciprocal_approx_fast(rc2[:], denf[:])
                    nc.vector.tensor_copy(rcb[:], rc2[:])
                    rbp = psf.tile([128, 512], f32, tag="pf", name="rb",
                                   bufs=2)
                    nc.tensor.matmul(
                        rbp[:], ind_sb[0:2, :], rcb[0:2, :],
                        start=True, stop=True,
                    )
                    nc.vector.tensor_mul(
                        atn[ot][jsl][:], atj[ot][jsl][:], rbp[:])

                def t_proj(jsl, sbi, osl):
                    pf = psf.tile([128, 512], f32, tag="pf", name="pf",
                                  bufs=2)
                    for ich in range(2):
                        nc.tensor.matmul(
                            pf[:],
                            atn[ich][jsl][:, sbi * 128:(sbi + 1) * 128],
                            wo_sb[:, ich, osl * 512:(osl + 1) * 512],
                            start=(ich == 0), stop=(ich == 1),
                        )
                    ob = nrm.tile([128, 512], bf16, tag="ob", name="ob",
                                  bufs=3)
                    nc.vector.tensor_copy(ob[:], pf[:])
                    sb = jsl * 4 + sbi
                    nc.sync.dma_start(
                        out_d[sb * 128:(sb + 1) * 128,
                              osl * 512:(osl + 1) * 512],
                        ob[:],
                    )

                # Rolling tail queue: normalization + output-projection work
                # drains at <=1 op per 2 units so the Act engine never
                # starves behind a burst of tail matmuls.
                from collections import deque
                pending_tail = deque()
                unit_ctr = 0
                for jsl in range(4):
                    nkb = 4 * (jsl + 1)
                    po_of_head = {}
                    for h in range(HPC):
                        po_of_head[h] = pso.tile([65, 512], f32, tag="po",
                                                 name=f"po{jsl}{h}", bufs=2)
                    # head-pair interleave: units alternate between the two
                    # heads of a pair, doubling the exp->PV slack
                    units = []
                    for hp in range(2):
                        hA, hB = 2 * hp, 2 * hp + 1
                        for g0 in range(0, nkb, 2):
                            units.append((hA, g0, *make_unit(jsl, hA, g0, nkb)))
                            units.append((hB, g0, *make_unit(jsl, hB, g0, nkb)))
                    prev = None
                    for u in units:
                        u[2]()  # emit_scores
                        if pending_tail and unit_ctr % 2 == 0:
                            pending_tail.popleft()()
                        unit_ctr += 1
                        if prev is not None:
                            ph, pg0 = prev[0], prev[1]
                            prev[3](po_of_head[ph])  # emit_pv
                            if pg0 + 2 >= nkb:
                                emit_stage(jsl, ph, po_of_head[ph])
                                if ph == 1:
                                    pending_tail.append(
                                        lambda j=jsl: t_norm(j, 0))
                        prev = u
                    ph, pg0 = prev[0], prev[1]
                    prev[3](po_of_head[ph])
                    emit_stage(jsl, ph, po_of_head[ph])
                    pending_tail.append(lambda j=jsl: t_norm(j, 1))
                    for sbi in range(4):
                        for osl in range(2):
                            pending_tail.append(
                                lambda j=jsl, s=sbi, o=osl: t_proj(j, s, o))
                # drain remaining tail work
                while pending_tail:
                    pending_tail.popleft()()
    nc.compile()
    return nc


def _host_prep(x, token_positions, WQ, WK, WV, WO):
    """Build the 8 per-core input maps."""
    pos = np.asarray(token_positions).astype(np.float32)
    k = np.arange(DK // 2, dtype=np.float32)
    inv_freq = 1.0 / (THETA ** (2.0 * k / DK))
    ang = pos[:, None] * inv_freq[None, :]          # [S, 32]
    c32 = np.cos(ang).T.astype(np.float32)          # [32, S]
    s32 = np.sin(ang).T.astype(np.float32)
    cosb = np.tile(c32, (4, 1)).astype(BF16)        # [128, S]
    sinb = np.concatenate([-s32, s32, -s32, s32], axis=0).astype(BF16)
    # 0/1 lower-triangle for zeroing the causal triangle of diagonal blocks
    kk = np.arange(128)[:, None]
    qq = np.arange(128)[None, :]
    tri = (qq >= kk).astype(np.float32).astype(BF16)        # [128, 128]
    # denominator-broadcast indicator: ind2[hl, r] = 1 iff r//64 == hl
    ind2 = np.zeros((2, 128), dtype=np.float32)
    ind2[0, 0:64] = 1.0
    ind2[1, 64:128] = 1.0
    ind2 = ind2.astype(BF16)

    perm = np.concatenate([np.arange(0, DK, 2), np.arange(1, DK, 2)])  # evens,odds

    in_maps = []
    for core in range(NCORES):
        b, hg = divmod(core, 4)
        ch0 = hg * 256
        qk_rows = np.concatenate([ch0 + hl * 64 + perm for hl in range(HPC)])
        def dev_w(w):  # [D, M] -> [128, 8, M] (contraction chunks)
            return np.ascontiguousarray(
                w.reshape(8, 128, -1).transpose(1, 0, 2)
            ).astype(BF16)

        xt = np.asarray(x[b]).T                       # [D, S]
        xt4 = np.ascontiguousarray(
            xt.reshape(8, 128, 4, 512).transpose(2, 1, 0, 3)
        ).astype(BF16)                                # [4, 128, 8, 512]
        in_maps.append({
            "xt": xt4,
            "wq": dev_w(np.asarray(WQ)[qk_rows, :].T),
            "wk": dev_w(np.asarray(WK)[qk_rows, :].T),
            "wv": dev_w(np.asarray(WV)[ch0:ch0 + 256, :].T),
            "wo": np.ascontiguousarray(
                np.asarray(WO)[:, ch0:ch0 + 256].T.reshape(2, 128, D)
                .transpose(1, 0, 2)
            ).astype(BF16),
            "cosb": cosb,
            "sinb": sinb,
            "tri": tri,
            "ind2": ind2,
        })
    return in_maps


LAST_EXEC_NS = None
LAST_RES = None


def kernel(x, token_positions, WQ, WK, WV, WO):
    global LAST_EXEC_NS, LAST_RES
    from concourse.bass_utils import run_bass_kernel_spmd

    if "nc" not in _COMPILED:
        _COMPILED["nc"] = _build_nc()
    nc = _COMPILED["nc"]

    in_maps = _host_prep(x, token_positions, WQ, WK, WV, WO)
    res = run_bass_kernel_spmd(nc, in_maps, list(range(NCORES)))
    LAST_RES = res
    LAST_EXEC_NS = res.exec_time_ns

    out = np.zeros((2, S, D), dtype=np.float32)
    for core in range(NCORES):
        out[core // 4] += np.asarray(res.results[core]["out"], dtype=np.float32)
    return out


# revision 23
# speedup vs baseline: 1.1941x; 1.0520x over previous
"""Trainium2 Bass kernel: causal multi-head self-attention with RoPE.

Problem: x[2,2048,1024], 16 heads, d_k=64, causal, RoPE(theta=1e4),
out = (softmax(rope(Q)rope(K)^T/8) V) WO^T.

Sharding (8 cores): data-parallel over batch (2) x head-parallel over
head groups (4 heads per core).  Each core computes Q/K/V projections
for its 4 heads, flash-style causal attention, and a partial output
projection over its 256 channels; the host sums the 4 partials per
batch element.

v4 design (head-pair-major, projection fillers):
  - All input DMAs ride one prioritized HW-DGE queue in consumption
    order (wq, x0, wk, x1, ...), so the first projection chain starts
    as soon as ~1.5MB has landed.
  - Phase A: Q/K projections for head-pair 0 (ot=0) + RoPE + V blocks
    0-3 only.
  - Phase B: attention for head-pair 0 across all 4 query slices.  The
    Act engine is the pacer (exp); the PE's idle slots are filled with
    the remaining projection work (V blocks 4-15, Q/K ot=1 chains,
    RoPE ot=1) drained from a deadline-ordered filler queue.
  - Phase C: attention for head-pair 1, with softmax normalization +
    output projection + stores rolling through the same filler
    mechanism.
  - exp skips fully-masked leading columns; the causal triangle of
    diagonal 128x128 blocks is zeroed multiplicatively post-exp on DVE.
  - psum->sbuf copies: Act during phase A (idle), DVE in B/C.
"""

import os
import sys
from collections import deque

for _p in ("/opt/trn_rl_repo",):
    if _p not in sys.path:
        sys.path.insert(0, _p)

import numpy as np
import ml_dtypes

BF16 = ml_dtypes.bfloat16

D = 1024
S = 2048
H = 16
DK = 64
HPC = 4          # heads per core
NCORES = 8
THETA = 10000.0

_COMPILED = {}


def _build_nc():
    import concourse.bass as bass  # noqa: F401
    import concourse.bacc as bacc
    import concourse.mybir as mybir
    import concourse.tile as tile

    bf16 = mybir.dt.bfloat16
    f32 = mybir.dt.float32
    Exp = mybir.ActivationFunctionType.Exp

    nc = bacc.Bacc(
        "TRN2", target_bir_lowering=False, debug=False, num_devices=NCORES
    )
    xt_d = nc.declare_dram_parameter("xt", [4, 128, 8, 512], bf16, isOutput=False)
    wq_d = nc.declare_dram_parameter("wq", [128, 8, 256], bf16, isOutput=False)
    wk_d = nc.declare_dram_parameter("wk", [128, 8, 256], bf16, isOutput=False)
    wv_d = nc.declare_dram_parameter("wv", [128, 8, 256], bf16, isOutput=False)
    wo_d = nc.declare_dram_parameter("wo", [128, 2, D], bf16, isOutput=False)
    cos_d = nc.declare_dram_parameter("cosb", [128, S], bf16, isOutput=False)
    sin_d = nc.declare_dram_parameter("sinb", [128, S], bf16, isOutput=False)
    tri_d = nc.declare_dram_parameter("tri", [128, 128], bf16, isOutput=False)
    ind_d = nc.declare_dram_parameter("ind2", [2, 128], bf16, isOutput=False)
    out_d = nc.declare_dram_parameter("out", [S, D], bf16, isOutput=True)

    with tile.TileContext(nc) as tc:
        with tc.tile_pool(name="const", bufs=1) as const:
            x_sb = [const.tile([128, 8, 512], bf16, name=f"x{i}") for i in range(4)]
            wq_sb = const.tile([128, 8, 256], bf16)
            wk_sb = const.tile([128, 8, 256], bf16)
            wv_sb = const.tile([128, 8, 256], bf16)
            wo_sb = const.tile([128, 2, D], bf16)
            cos_sb = const.tile([128, S], bf16)
            sin_sb = const.tile([128, S], bf16)
            tri_sb = const.tile([128, 128], bf16)
            ind_sb = const.tile([2, 128], bf16)
            v_sb = const.tile([128, 16, 4, 65], bf16)
            qraw = [const.tile([128, S], bf16, name=f"qraw{i}") for i in range(2)]
            kraw = [const.tile([128, S], bf16, name=f"kraw{i}") for i in range(2)]
            qrot = [const.tile([128, S], bf16, name=f"qrot{i}") for i in range(2)]
            krot = [const.tile([128, S], bf16, name=f"krot{i}") for i in range(2)]
            # unnormalized / normalized head outputs, per (ot, jsl)
            atj = [[const.tile([128, 512], bf16, name=f"at{o}_{j}")
                    for j in range(4)] for o in range(2)]
            atn = [[const.tile([128, 512], bf16, name=f"an{o}_{j}")
                    for j in range(4)] for o in range(2)]
            den_sb = const.tile([2, 8, 512], bf16)  # [hl, jsl*2+ot, q]

            # All inputs in-order on the single HW-DGE (sync) queue so the
            # full HBM bandwidth goes to each in consumption order.
            nc.sync.dma_start(wq_sb[:], wq_d[:])
            nc.sync.dma_start(x_sb[0][:], xt_d[0])
            nc.sync.dma_start(wk_sb[:], wk_d[:])
            nc.sync.dma_start(x_sb[1][:], xt_d[1])
            nc.sync.dma_start(x_sb[2][:], xt_d[2])
            nc.sync.dma_start(cos_sb[:], cos_d[:])
            nc.sync.dma_start(sin_sb[:], sin_d[:])
            nc.sync.dma_start(x_sb[3][:], xt_d[3])
            nc.sync.dma_start(wv_sb[:], wv_d[:])
            nc.sync.dma_start(tri_sb[:], tri_d[:])
            nc.sync.dma_start(ind_sb[:], ind_d[:])
            nc.sync.dma_start(wo_sb[:], wo_d[:])
            nc.vector.memset(v_sb[:, :, :, 64:65], 1.0)

            with tc.tile_pool(name="rope", bufs=1) as rp:

                def rope_pair(raw, rot, ot):
                    sw = rp.tile([128, S], bf16, tag="sw", name="sw", bufs=2)
                    t1 = rp.tile([128, S], bf16, tag="t1", name="t1", bufs=2)
                    for blk in range(4):
                        src = blk ^ 1
                        nc.sync.dma_start(
                            sw[blk * 32:(blk + 1) * 32, :],
                            raw[ot][src * 32:(src + 1) * 32, :],
                        )
                    nc.vector.tensor_mul(t1[:], raw[ot][:], cos_sb[:])
                    nc.vector.tensor_mul(sw[:], sw[:], sin_sb[:])
                    nc.vector.tensor_add(rot[ot][:], t1[:], sw[:])

                def qk_chain(ot, nsl, w_sb, raw, ps, act_copy):
                    for c in range(8):
                        nc.tensor.matmul(
                            ps[:],
                            w_sb[:, c, ot * 128:(ot + 1) * 128],
                            x_sb[nsl][:, c, :],
                            start=(c == 0), stop=(c == 7),
                        )
                    dst = raw[ot][:, nsl * 512:(nsl + 1) * 512]
                    if act_copy:
                        nc.scalar.copy(dst, ps[:])
                    else:
                        nc.vector.tensor_copy(dst, ps[:])

                def v_chain(sb, ps, act_copy):
                    for c in range(8):
                        nc.tensor.matmul(
                            ps[:, 0:256],
                            x_sb[sb // 4][:, c,
                                          (sb % 4) * 128:(sb % 4 + 1) * 128],
                            wv_sb[:, c, :],
                            start=(c == 0), stop=(c == 7),
                        )
                    src = ps[:, 0:256].rearrange("p (h d) -> p h d", h=4)
                    if act_copy:
                        nc.scalar.copy(v_sb[:, sb, :, 0:64], src)
                    else:
                        nc.vector.tensor_copy(v_sb[:, sb, :, 0:64], src)

                # ---- phase A ----
                with tc.tile_pool(name="pj", bufs=1, space="PSUM") as pjp:
                    for nsl in range(4):
                        for w_sb, raw in ((wq_sb, qraw), (wk_sb, kraw)):
                            ps = pjp.tile([128, 512], f32, tag="pj",
                                          name="pj", bufs=4)
                            qk_chain(0, nsl, w_sb, raw, ps, act_copy=True)
                    rope_pair(qraw, qrot, 0)
                    rope_pair(kraw, krot, 0)
                    for sb in range(4):
                        ps = pjp.tile([128, 512], f32, tag="pj", name="pv",
                                      bufs=4)
                        v_chain(sb, ps, act_copy=True)

                # ---- phases B/C ----
                with tc.tile_pool(name="ps_s", bufs=1, space="PSUM") as psc, \
                     tc.tile_pool(name="ps_o", bufs=1, space="PSUM") as pso, \
                     tc.tile_pool(name="ps_f", bufs=1, space="PSUM") as psf, \
                     tc.tile_pool(name="pp", bufs=1) as ppool, \
                     tc.tile_pool(name="nrm", bufs=1) as nrm:

                    def make_unit(jsl, h, g0, nkb):
                        ot, hl = divmod(h, 2)
                        r0 = hl * 64
                        qr, kr = qrot[ot], krot[ot]
                        state = {}

                        def emit_scores():
                            sp = psc.tile([128, 1024], f32, tag="sc",
                                          name="sp", bufs=2)
                            pt = ppool.tile([128, 1024], bf16, tag="pt",
                                            name="pt", bufs=3)
                            state["pt"] = pt
                            dgs = []
                            for i in range(2):
                                kb = g0 + i
                                dg = kb - 4 * jsl
                                c0 = dg * 128 if dg > 0 else 0
                                nc.tensor.matmul(
                                    sp[:, i * 512 + c0:(i + 1) * 512],
                                    kr[r0:r0 + 64, kb * 128:(kb + 1) * 128],
                                    qr[r0:r0 + 64,
                                       jsl * 512 + c0:(jsl + 1) * 512],
                                    start=True, stop=True,
                                )
                                dgs.append(dg)
                            # exp: skip fully-masked leading columns of the
                            # deep diagonal blocks
                            if dgs[0] >= 2:
                                for i in range(2):
                                    c0 = dgs[i] * 128
                                    nc.scalar.activation(
                                        pt[:, i * 512 + c0:(i + 1) * 512],
                                        sp[:, i * 512 + c0:(i + 1) * 512],
                                        Exp, scale=0.125,
                                    )
                            else:
                                nc.scalar.activation(
                                    pt[:, 0:1024], sp[:, 0:1024],
                                    Exp, scale=0.125,
                                )
                            # zero the causal triangle of diagonal blocks
                            for i in range(2):
                                dg = g0 + i - 4 * jsl
                                if 0 <= dg <= 3:
                                    a = i * 512 + dg * 128
                                    nc.vector.tensor_mul(
                                        pt[:, a:a + 128], pt[:, a:a + 128],
                                        tri_sb[:],
                                    )

                        def emit_pv(po):
                            pt = state["pt"]
                            for i in range(2):
                                kb = g0 + i
                                dg = kb - 4 * jsl
                                c0 = dg * 128 if dg > 0 else 0
                                nc.tensor.matmul(
                                    po[:, c0:512],
                                    v_sb[:, kb, h, 0:65],
                                    pt[:, i * 512 + c0:(i + 1) * 512],
                                    start=(kb == 0), stop=(kb == nkb - 1),
                                )

                        return emit_scores, emit_pv

                    def emit_stage(jsl, h, po):
                        ot, hl = divmod(h, 2)
                        r0 = hl * 64
                        tm = nrm.tile([65, 512], bf16, tag="tm", name="tm",
                                      bufs=3)
                        nc.vector.tensor_copy(tm[:], po[:])
                        nc.sync.dma_start(
                            atj[ot][jsl][r0:r0 + 64, :], tm[0:64, :])
                        nc.sync.dma_start(
                            den_sb[hl:hl + 1, jsl * 2 + ot, :], tm[64:65, :])

                    def t_norm(jsl, ot):
                        denf = nrm.tile([2, 512], f32, tag="denf",
                                        name="denf", bufs=2)
                        rc2 = nrm.tile([2, 512], f32, tag="rc2", name="rc2",
                                       bufs=2)
                        rcb = nrm.tile([2, 512], bf16, tag="rcb", name="rcb",
                                       bufs=2)
                        nc.vector.tensor_copy(
                            denf[:], den_sb[0:2, jsl * 2 + ot, :])
                        nc.vector.reciprocal_approx_fast(rc2[:], denf[:])
                        nc.vector.tensor_copy(rcb[:], rc2[:])
                        rbp = psf.tile([128, 512], f32, tag="pf", name="rb",
                                       bufs=2)
                        nc.tensor.matmul(
                            rbp[:], ind_sb[0:2, :], rcb[0:2, :],
                            start=True, stop=True,
                        )
                        nc.vector.tensor_mul(
                            atn[ot][jsl][:], atj[ot][jsl][:], rbp[:])

                    def t_proj(jsl, sbi, osl):
                        pf = psf.tile([128, 512], f32, tag="pf", name="pf",
                                      bufs=2)
                        for ich in range(2):
                            nc.tensor.matmul(
                                pf[:],
                                atn[ich][jsl][:, sbi * 128:(sbi + 1) * 128],
                                wo_sb[:, ich, osl * 512:(osl + 1) * 512],
                                start=(ich == 0), stop=(ich == 1),
                            )
                        ob = nrm.tile([128, 512], bf16, tag="ob", name="ob",
                                      bufs=3)
                        nc.vector.tensor_copy(ob[:], pf[:])
                        sb = jsl * 4 + sbi
                        nc.sync.dma_start(
                            out_d[sb * 128:(sb + 1) * 128,
                                  osl * 512:(osl + 1) * 512],
                            ob[:],
                        )

                    def f_vchain(sb):
                        ps = psf.tile([128, 512], f32, tag="pf", name="pv",
                                      bufs=2)
                        v_chain(sb, ps, act_copy=False)

                    def f_qkchain(nsl, which):
                        w_sb, raw = ((wq_sb, qraw), (wk_sb, kraw))[which]
                        ps = psf.tile([128, 512], f32, tag="pf", name="pj",
                                      bufs=2)
                        qk_chain(1, nsl, w_sb, raw, ps, act_copy=False)

                    # deadline-ordered filler queue for phase B, then the
                    # rolling normalization/projection tail for phase C
                    fillers = deque()
                    for sb in range(4, 8):
                        fillers.append(lambda s=sb: f_vchain(s))
                    for nsl in range(2):
                        for w in range(2):
                            fillers.append(lambda n=nsl, w_=w: f_qkchain(n, w_))
                    for sb in range(8, 12):
                        fillers.append(lambda s=sb: f_vchain(s))
                    for nsl in range(2, 4):
                        for w in range(2):
                            fillers.append(lambda n=nsl, w_=w: f_qkchain(n, w_))
                    for sb in range(12, 16):
                        fillers.append(lambda s=sb: f_vchain(s))
                    fillers.append(lambda: rope_pair(qraw, qrot, 1))
                    fillers.append(lambda: rope_pair(kraw, krot, 1))

                    fill_credit = [0.0]

                    def run_stream(pair, rate):
                        """Emit the attention stream for one head pair across
                        all 4 query slices, draining `fillers` at `rate` ops
                        per unit into the PE's Act-bound idle slots."""
                        hA, hB = 2 * pair, 2 * pair + 1
                        prev = None
                        for jsl in range(4):
                            nkb = 4 * (jsl + 1)
                            po = {
                                hA: pso.tile([65, 512], f32, tag="po",
                                             name=f"poA{pair}{jsl}", bufs=2),
                                hB: pso.tile([65, 512], f32, tag="po",
                                             name=f"poB{pair}{jsl}", bufs=2),
                            }
                            units = []
                            for g0 in range(0, nkb, 2):
                                units.append(
                                    (jsl, hA, g0, po, *make_unit(jsl, hA, g0, nkb)))
                                units.append(
                                    (jsl, hB, g0, po, *make_unit(jsl, hB, g0, nkb)))
                            for u in units:
                                u[4]()  # emit_scores
                                fill_credit[0] += rate
                                while fillers and fill_credit[0] >= 1.0:
                                    fill_credit[0] -= 1.0
                                    fillers.popleft()()
                                if prev is not None:
                                    pjsl, ph, pg0, ppo = prev[0], prev[1], prev[2], prev[3]
                                    prev[5](ppo[ph])  # emit_pv
                                    if pg0 + 2 >= 4 * (pjsl + 1):
                                        emit_stage(pjsl, ph, ppo[ph])
                                        if ph % 2 == 1:
                                            fillers.append(
                                                lambda j=pjsl, o=ph // 2:
                                                t_norm(j, o))
                                            if ph >= 2:
                                                for sbi in range(4):
                                                    for osl in range(2):
                                                        fillers.append(
                                                            lambda j=pjsl,
                                                            s=sbi, o=osl:
                                                            t_proj(j, s, o))
                                prev = u
                        return prev

                    prev = run_stream(0, rate=0.67)
                    # flush the last pair-0 unit before pair 1 begins
                    pjsl, ph, pg0, ppo = prev[0], prev[1], prev[2], prev[3]
                    prev[5](ppo[ph])
                    emit_stage(pjsl, ph, ppo[ph])
                    fillers.append(lambda j=pjsl: t_norm(j, 0))

                    prev = run_stream(1, rate=1.0)
                    pjsl, ph, pg0, ppo = prev[0], prev[1], prev[2], prev[3]
                    prev[5](ppo[ph])
                    emit_stage(pjsl, ph, ppo[ph])
                    fillers.append(lambda j=pjsl: t_norm(j, 1))
                    for sbi in range(4):
                        for osl in range(2):
                            fillers.append(
                                lambda j=pjsl, s=sbi, o=osl: t_proj(j, s, o))
                    while fillers:
                        fillers.popleft()()
    nc.compile()
    return nc


def _host_prep(x, token_positions, WQ, WK, WV, WO):
    """Build the 8 per-core input maps."""
    pos = np.asarray(token_positions).astype(np.float32)
    k = np.arange(DK // 2, dtype=np.float32)
    inv_freq = 1.0 / (THETA ** (2.0 * k / DK))
    ang = pos[:, None] * inv_freq[None, :]          # [S, 32]
    c32 = np.cos(ang).T.astype(np.float32)          # [32, S]
    s32 = np.sin(ang).T.astype(np.float32)
    cosb = np.tile(c32, (4, 1)).astype(BF16)        # [128, S]
    sinb = np.concatenate([-s32, s32, -s32, s32], axis=0).astype(BF16)
    # 0/1 lower-triangle for zeroing the causal triangle of diagonal blocks
    kk = np.arange(128)[:, None]
    qq = np.arange(128)[None, :]
    tri = (qq >= kk).astype(np.float32).astype(BF16)        # [128, 128]
    # denominator-broadcast indicator: ind2[hl, r] = 1 iff r//64 == hl
    ind2 = np.zeros((2, 128), dtype=np.float32)
    ind2[0, 0:64] = 1.0
    ind2[1, 64:128] = 1.0
    ind2 = ind2.astype(BF16)

    perm = np.concatenate([np.arange(0, DK, 2), np.arange(1, DK, 2)])  # evens,odds

    in_maps = []
    for core in range(NCORES):
        b, hg = divmod(core, 4)
        ch0 = hg * 256
        qk_rows = np.concatenate([ch0 + hl * 64 + perm for hl in range(HPC)])
        def dev_w(w):  # [D, M] -> [128, 8, M] (contraction chunks)
            return np.ascontiguousarray(
                w.reshape(8, 128, -1).transpose(1, 0, 2)
            ).astype(BF16)

        xt = np.asarray(x[b]).T                       # [D, S]
        xt4 = np.ascontiguousarray(
            xt.reshape(8, 128, 4, 512).transpose(2, 1, 0, 3)
        ).astype(BF16)                                # [4, 128, 8, 512]
        in_maps.append({
            "xt": xt4,
            "wq": dev_w(np.asarray(WQ)[qk_rows, :].T),
            "wk": dev_w(np.asarray(WK)[qk_rows, :].T),
            "wv": dev_w(np.asarray(WV)[ch0:ch0 + 256, :].T),
            "wo": np.ascontiguousarray(
                np.asarray(WO)[:, ch0:ch0 + 256].T.reshape(2, 128, D)
                .transpose(1, 0, 2)
            ).astype(BF16),
            "cosb": cosb,
            "sinb": sinb,
            "tri": tri,
            "ind2": ind2,
        })
    return in_maps


LAST_EXEC_NS = None
LAST_RES = None


def kernel(x, token_positions, WQ, WK, WV, WO):
    global LAST_EXEC_NS, LAST_RES
    from concourse.bass_utils import run_bass_kernel_spmd

    if "nc" not in _COMPILED:
        _COMPILED["nc"] = _build_nc()
    nc = _COMPILED["nc"]

    in_maps = _host_prep(x, token_positions, WQ, WK, WV, WO)
    res = run_bass_kernel_spmd(nc, in_maps, list(range(NCORES)))
    LAST_RES = res
    LAST_EXEC_NS = res.exec_time_ns

    out = np.zeros((2, S, D), dtype=np.float32)
    for core in range(NCORES):
        out[core // 4] += np.asarray(res.results[core]["out"], dtype=np.float32)
    return out
